# revision 11
# baseline (speedup 1.0000x reference)
"""2-layer GAT (GATConv x2, PyG-style with self-loops) on 8 Trainium2 NeuronCores.

Strategy (graph data-parallel, 1-D partition by destination node):
  - Nodes (and their incoming edges) are sharded across the 8 cores.
  - Each core redundantly computes the dense per-node tables
    h_ext = [h | alo_src | alo_dst] for ALL nodes (cheap, streaming), then
    processes its own destination bins:
      * edges sorted by dst, packed into bins of <=15 contiguous dst nodes /
        128 edge slots (self-loops included; each node's designated self-loop
        slot carries alo_dst for the whole bin),
      * per bin one 128-row indirect DMA gathers h_ext[src],
      * attention exp(leaky_relu(es+ed)) on ACT, selector matmuls on PE do the
        segment softmax numerator/denominator reduction,
      * normalized output rows scattered to the core's output shard.
  - Layer boundary: host concatenates the relu'd layer-1 shards (bf16) and
    launches layer 2 (h2 tables via DMA-transpose matmul, same bins).

The graph edge tables are built on the host from the actual edge_index input;
everything runs through bass_utils.run_bass_kernel_spmd (PJRT/axon path).
"""
import os
import sys

sys.path.insert(0, '/opt/trn_rl_repo')

import numpy as np
import ml_dtypes

import concourse.bass as bass
import concourse.tile as tile
from concourse import bacc, mybir
from concourse.bass_utils import run_bass_kernel_spmd

_TRACE = bool(os.environ.get("GAT_TRACE"))
LAST_EXEC_NS = []  # exec_time_ns per launch when _TRACE is set


def _install_ntff_hook():
    """Provide the antenv.axon_hooks module run_bass_kernel_spmd(trace=True)
    needs, driving NTFF profiling via ctypes into libaxon_pjrt.so."""
    import types, ctypes, contextlib
    so_path = "/opt/axon/libaxon_pjrt.so"
    lib = ctypes.CDLL(so_path)
    if not hasattr(lib, "axon_start_nrt_profile"):
        return False
    lib.axon_start_nrt_profile.argtypes = [ctypes.POINTER(ctypes.c_int64),
                                           ctypes.c_size_t]
    lib.axon_start_nrt_profile.restype = ctypes.c_int64
    lib.axon_stop_nrt_profile.argtypes = [ctypes.c_char_p]
    lib.axon_stop_nrt_profile.restype = ctypes.c_int64

    @contextlib.contextmanager
    def _hook(output_dir, device_ids):
        import jax
        jax.devices()
        if device_ids:
            ids = (ctypes.c_int64 * len(device_ids))(*device_ids)
            rc = lib.axon_start_nrt_profile(ids, len(device_ids))
        else:
            rc = lib.axon_start_nrt_profile(None, 0)
        if rc != 0:
            raise RuntimeError(f"axon_start_nrt_profile rc={rc}")
        try:
            yield
        finally:
            lib.axon_stop_nrt_profile(str(output_dir).encode())

    mod = types.ModuleType("antenv.axon_hooks")
    mod.get_axon_ntff_profile_hook = lambda: _hook
    mod.set_axon_ntff_profile_hook = lambda h: None
    sys.modules["antenv.axon_hooks"] = mod
    from concourse import bass_utils
    bass_utils.upload_artifacts = lambda tmpdir: f"local:{tmpdir}"
    return True


if _TRACE:
    _install_ntff_hook()


def _run(nc, in_maps, core_ids):
    res = run_bass_kernel_spmd(nc, in_maps, core_ids, trace=_TRACE)
    if _TRACE:
        LAST_EXEC_NS.append(res.exec_time_ns)
    return res

F32 = mybir.dt.float32
BF16 = mybir.dt.bfloat16
I32 = mybir.dt.int32

N_CORES = 8
CAP_E = 128      # edge slots per bin
CAP_N = 15      # max real nodes per bin (slot 15 = trash)
GRP = 8          # bins per group (batched selector generation / scatter)
NEG_SLOPE = 0.2


# ----------------------------------------------------------------------------
# host-side graph preprocessing
# ----------------------------------------------------------------------------

def _build_tables(src, dst, n_nodes, n_cores):
    """Per-core bin tables. Edges (src, dst) int32 WITHOUT self-loops; they are
    added here (one per node, marked as the alo_dst carrier)."""
    S = n_nodes // n_cores
    per_core = []
    for c in range(n_cores):
        lo, hi = c * S, (c + 1) * S
        m = (dst >= lo) & (dst < hi)
        es, ed = src[m], dst[m] - lo
        # append designated self-loops
        es = np.concatenate([es, np.arange(lo, hi, dtype=np.int32)])
        marker = np.zeros(es.shape[0], np.bool_)
        marker[ed.shape[0]:] = True
        ed = np.concatenate([ed, np.arange(S, dtype=np.int32)])
        order = np.argsort(ed, kind='stable')
        es, ed, marker = es[order], ed[order], marker[order]
        deg = np.bincount(ed, minlength=S)  # >=1 everywhere
        # greedy contiguous packing: <=CAP_E edges, <=CAP_N nodes per bin
        bin_of_node = np.empty(S, np.int32)
        first_node = []
        n0 = 0
        while n0 < S:
            cnt = 0
            edges = 0
            while (n0 + cnt < S and cnt < CAP_N
                   and edges + deg[n0 + cnt] <= CAP_E):
                edges += deg[n0 + cnt]
                cnt += 1
            assert cnt > 0, f"degree {deg[n0]} exceeds bin capacity"
            bin_of_node[n0:n0 + cnt] = len(first_node)
            first_node.append(n0)
            n0 += cnt
        first_node = np.asarray(first_node, np.int32)
        B = len(first_node)
        # per-edge slot positions
        ebin = bin_of_node[ed]
        edge_off = np.zeros(B + 1, np.int64)
        np.add.at(edge_off[1:], ebin, 1)
        np.cumsum(edge_off, out=edge_off)
        slot = np.arange(es.shape[0], dtype=np.int64) - edge_off[ebin]
        srcT = np.zeros((B, CAP_E), np.int32)
        lidT = np.full((B, CAP_E), CAP_N, np.float32)
        slidT = np.full((B, CAP_E), 16, np.float32)
        srcT[ebin, slot] = es
        lidT[ebin, slot] = (ed - first_node[ebin]).astype(np.float32)
        slidT[ebin[marker], slot[marker]] = (ed - first_node[ebin])[marker]
        outT = np.full((B, 16), S, np.int32)
        nb = np.diff(np.append(first_node, S))
        for b in range(B):
            outT[b, :nb[b]] = first_node[b] + np.arange(nb[b])
        per_core.append((srcT, lidT, slidT, outT))

    B = max(t[0].shape[0] for t in per_core)
    B = -(-B // GRP) * GRP
    G = B // GRP
    out = {k: [] for k in ("srcP", "outP", "lidsl", "lidR")}
    for (srcT, lidT, slidT, outT) in per_core:
        b0 = srcT.shape[0]
        srcT = np.concatenate([srcT, np.zeros((B - b0, CAP_E), np.int32)])
        lidT = np.concatenate([lidT, np.full((B - b0, CAP_E), CAP_N, np.float32)])
        slidT = np.concatenate([slidT, np.full((B - b0, CAP_E), 16, np.float32)])
        outT = np.concatenate([outT, np.full((B - b0, 16), S, np.int32)])
        # srcP: [G, 128, 8] (slot-major), outP: [G, 128]
        out["srcP"].append(srcT.reshape(G, GRP, CAP_E).transpose(0, 2, 1).copy())
        out["outP"].append(outT.reshape(G, GRP * 16))
        out["lidsl"].append(np.concatenate(
            [lidT.reshape(G, GRP, CAP_E).transpose(0, 2, 1),
             slidT.reshape(G, GRP, CAP_E).transpose(0, 2, 1)], axis=2).copy())
        out["lidR"].append(lidT.reshape(G, 1, GRP * CAP_E).copy())
    return out, B, G, S


# ----------------------------------------------------------------------------
# device programs
# ----------------------------------------------------------------------------

def _phase_bins(nc, tc, consts, tabs, h_tab, out_tab, G, n_heads, hd, b_rep,
                out_dtype, relu):
    """Shared bin-processing phase. h_tab rows = [h(nh) | es(C) | ed(C)];
    out rows = nh floats (+bias, optional relu)."""
    C = n_heads
    nh = n_heads * hd
    W = nh + 2 * C  # gathered row width
    iota16T, iotaP16, sel8 = consts["iota16T"], consts["iotaP16"], consts["sel8"]
    srcP, outP, lidsl, lidR = tabs["srcP"], tabs["outP"], tabs["lidsl"], tabs["lidR"]

    with tc.tile_pool(name="bsb", bufs=3) as sb, \
         tc.tile_pool(name="bps", bufs=2, space="PSUM") as ps:
        ones16 = sb.tile([1, 16], F32, tag="ones16")
        nc.vector.memset(ones16[:], 1.0)
        for g in range(G):
            srcT = sb.tile([128, GRP], I32, tag="srcT")
            nc.sync.dma_start(out=srcT[:], in_=srcP[g])
            outT = sb.tile([128, 1], I32, tag="outT")
            nc.sync.dma_start(out=outT[:], in_=outP[g][:, None])
            lidsl_t = sb.tile([128, 16], F32, tag="lidsl")
            nc.sync.dma_start(out=lidsl_t[:], in_=lidsl[g])
            lidR_t = sb.tile([1, GRP * CAP_E], F32, tag="lidR")
            nc.sync.dma_start(out=lidR_t[:], in_=lidR[g])

            S8 = sb.tile([128, 128], F32, tag="S8")
            nc.vector.tensor_tensor(
                out=S8[:].rearrange("p (b j) -> p b j", j=16),
                in0=lidsl_t[:, 0:8, None].to_broadcast([128, 8, 16]),
                in1=iota16T[:].rearrange("p (b j) -> p b j", j=16),
                op=mybir.AluOpType.is_equal)
            Sf8 = sb.tile([128, 128], F32, tag="Sf8")
            nc.vector.tensor_tensor(
                out=Sf8[:].rearrange("p (b j) -> p b j", j=16),
                in0=lidsl_t[:, 8:16, None].to_broadcast([128, 8, 16]),
                in1=iota16T[:].rearrange("p (b j) -> p b j", j=16),
                op=mybir.AluOpType.is_equal)
            alo_ps = ps.tile([16, 8 * C], F32, tag="alo")
            edp = ps.tile([128, 8 * C], F32, tag="edp")
            stag = sb.tile([128, nh], out_dtype, tag="stag")
            for b in range(GRP):
                gb = sb.tile([128, W], F32, tag="gb")
                nc.gpsimd.indirect_dma_start(
                    out=gb[:], out_offset=None, in_=h_tab[:],
                    in_offset=bass.IndirectOffsetOnAxis(
                        ap=srcT[:, b:b + 1], axis=0))
                nc.tensor.matmul(out=alo_ps[:, b * C:(b + 1) * C],
                                 lhsT=Sf8[:, b * 16:(b + 1) * 16],
                                 rhs=gb[:, nh + C: nh + 2 * C],
                                 start=True, stop=True)
                alo_sb = sb.tile([16, C], F32, tag="alo_sb")
                nc.vector.tensor_copy(out=alo_sb[:],
                                      in_=alo_ps[:, b * C:(b + 1) * C])
                rep = ps.tile([16, CAP_E], F32, tag="rep")
                nc.tensor.matmul(out=rep[:], lhsT=ones16[:],
                                 rhs=lidR_t[:, b * CAP_E:(b + 1) * CAP_E],
                                 start=True, stop=True)
                STb = sb.tile([16, CAP_E], F32, tag="STb")
                nc.vector.tensor_tensor(out=STb[:], in0=rep[:],
                                        in1=iotaP16[0:16, :],
                                        op=mybir.AluOpType.is_equal)
                nc.tensor.matmul(out=edp[:, b * C:(b + 1) * C],
                                 lhsT=STb[:],
                                 rhs=alo_sb[:], start=True, stop=True)
                ex = sb.tile([128, C], F32, tag="ex")
                nc.vector.tensor_tensor(out=ex[:], in0=gb[:, nh:nh + C],
                                        in1=edp[:, b * C:(b + 1) * C],
                                        op=mybir.AluOpType.add)
                exs = sb.tile([128, C], F32, tag="exs")
                nc.vector.tensor_scalar_mul(out=exs[:], in0=ex[:],
                                            scalar1=NEG_SLOPE)
                nc.vector.tensor_tensor(out=ex[:], in0=ex[:], in1=exs[:],
                                        op=mybir.AluOpType.max)
                nc.scalar.activation(out=ex[:], in_=ex[:],
                                     func=mybir.ActivationFunctionType.Exp)
                rhs = sb.tile([128, nh + C], F32, tag="rhs")
                nc.vector.tensor_tensor(
                    out=rhs[:, 0:nh].rearrange("p (h d) -> p h d", d=hd),
                    in0=gb[:, 0:nh].rearrange("p (h d) -> p h d", d=hd),
                    in1=ex[:, :, None].to_broadcast([128, C, hd]),
                    op=mybir.AluOpType.mult)
                nc.vector.tensor_copy(out=rhs[:, nh:nh + C], in_=ex[:])
                grp = ps.tile([16, nh + C], F32, tag="grp")
                nc.tensor.matmul(out=grp[:],
                                 lhsT=S8[:, b * 16:(b + 1) * 16],
                                 rhs=rhs[:], start=True, stop=True)
                # bin epilogue: normalize + bias (+relu) at base partition 0
                recip = sb.tile([16, C], F32, tag="recip")
                nc.vector.reciprocal(out=recip[:], in_=grp[:, nh:nh + C])
                t1 = sb.tile([16, nh], F32, tag="t1")
                nc.vector.tensor_tensor(
                    out=t1[:].rearrange("p (h d) -> p h d", d=hd),
                    in0=grp[:, 0:nh].rearrange("p (h d) -> p h d", d=hd),
                    in1=recip[:, :, None].to_broadcast([16, C, hd]),
                    op=mybir.AluOpType.mult)
                bstag = sb.tile([16, nh], out_dtype, tag="bstag")
                if relu:
                    nc.vector.tensor_tensor(out=t1[:], in0=t1[:],
                                            in1=b_rep[0:16, :],
                                            op=mybir.AluOpType.add)
                    nc.vector.tensor_scalar_max(out=bstag[:], in0=t1[:],
                                                scalar1=0.0)
                else:
                    nc.vector.tensor_tensor(out=bstag[:], in0=t1[:],
                                            in1=b_rep[0:16, :],
                                            op=mybir.AluOpType.add)
                nc.sync.dma_start(out=stag[b * 16:(b + 1) * 16, :],
                                  in_=bstag[:])
            nc.gpsimd.indirect_dma_start(
                out=out_tab[:], out_offset=bass.IndirectOffsetOnAxis(
                    ap=outT[:], axis=0),
                in_=stag[:], in_offset=None)


def _make_consts(nc, tc, pool):
    iota16T = pool.tile([128, 128], F32, tag="iota16T")
    nc.vector.iota(iota16T[:].rearrange("p (b j) -> p b j", j=16),
                   pattern=[[0, 8], [1, 16]], base=0, channel_multiplier=0,
                   allow_small_or_imprecise_dtypes=True)
    iotaP16 = pool.tile([128, 128], F32, tag="iotaP16")
    nc.vector.iota(iotaP16[:].rearrange("p (b j) -> p b j", j=16),
                   pattern=[[0, 8], [0, 16]], base=0, channel_multiplier=1,
                   allow_small_or_imprecise_dtypes=True)
    # iotaP16 needs p%16; channel_multiplier gives p. fix: subtract 16*(p//16)
    # simpler: generate base p then compare against rep which is lid in 0..15
    # -> instead generate p%16 via iota on 3d pattern with channel blocks:
    return {"iota16T": iota16T, "iotaP16": iotaP16}


def build_l1(shapes):
    n_nodes_pad, G, S = shapes["n_pad"], shapes["G"], shapes["S"]
    T = n_nodes_pad // 128
    nc = bacc.Bacc(None)
    xt = nc.declare_dram_parameter("xt", [T, 128, 128], F32, isOutput=False)
    W1 = nc.declare_dram_parameter("W1", [128, 64], F32, isOutput=False)
    asrc = nc.declare_dram_parameter("asrc", [128, 64], F32, isOutput=False)
    adst = nc.declare_dram_parameter("adst", [128, 64], F32, isOutput=False)
    b1r = nc.declare_dram_parameter("b1r", [128, 64], F32, isOutput=False)
    iota16T_d = nc.declare_dram_parameter("iota16T", [128, 128], F32, isOutput=False)
    iotaP16_d = nc.declare_dram_parameter("iotaP16", [128, 128], F32, isOutput=False)
    sel8_d = nc.declare_dram_parameter("sel8", [8, 128], F32, isOutput=False)
    srcP = nc.declare_dram_parameter("srcP", [G, 128, GRP], I32, isOutput=False)
    outP = nc.declare_dram_parameter("outP", [G, 128], I32, isOutput=False)
    lidsl = nc.declare_dram_parameter("lidsl", [G, 128, 16], F32, isOutput=False)
    lidR = nc.declare_dram_parameter("lidR", [G, 1, GRP * CAP_E], F32, isOutput=False)
    out1 = nc.declare_dram_parameter("out1", [S + 16, 64], BF16, isOutput=True)
    h_ext = nc.dram_tensor("h_ext", [n_nodes_pad, 80], F32)

    with tile.TileContext(nc) as tc:
        with tc.tile_pool(name="const", bufs=1) as cpool:
            W1sb = cpool.tile([128, 64], F32, tag="W1sb")
            nc.sync.dma_start(out=W1sb[:], in_=W1[:])
            asb = cpool.tile([128, 64], F32, tag="asb")
            nc.sync.dma_start(out=asb[:], in_=asrc[:])
            dsb = cpool.tile([128, 64], F32, tag="dsb")
            nc.sync.dma_start(out=dsb[:], in_=adst[:])
            bsb = cpool.tile([128, 64], F32, tag="bsb")
            nc.sync.dma_start(out=bsb[:], in_=b1r[:])
            iota16T = cpool.tile([128, 128], F32, tag="i16")
            nc.sync.dma_start(out=iota16T[:], in_=iota16T_d[:])
            iotaP16 = cpool.tile([128, 128], F32, tag="iP16")
            nc.sync.dma_start(out=iotaP16[:], in_=iotaP16_d[:])
            sel8 = cpool.tile([8, 128], F32, tag="sel8")
            nc.sync.dma_start(out=sel8[:], in_=sel8_d[:])

            # phase A: h_ext = [x@W1 | alo_s | alo_d]
            with tc.tile_pool(name="pa", bufs=3) as pa, \
                 tc.tile_pool(name="pap", bufs=3, space="PSUM") as pap:
                for t in range(T):
                    xt_t = pa.tile([128, 128], F32, tag="xt")
                    nc.sync.dma_start(out=xt_t[:], in_=xt[t])
                    hp = pap.tile([128, 64], F32, tag="hp")
                    nc.tensor.matmul(out=hp[:], lhsT=xt_t[:], rhs=W1sb[:],
                                     start=True, stop=True)
                    he = pa.tile([128, 80], F32, tag="he")
                    nc.vector.tensor_copy(out=he[:, 0:64], in_=hp[:])
                    tmp = pa.tile([128, 64], F32, tag="tmp")
                    nc.vector.tensor_tensor(out=tmp[:], in0=hp[:], in1=asb[:],
                                            op=mybir.AluOpType.mult)
                    nc.vector.tensor_reduce(
                        out=he[:, 64:72],
                        in_=tmp[:].rearrange("p (h d) -> p h d", d=8),
                        axis=mybir.AxisListType.X, op=mybir.AluOpType.add)
                    nc.vector.tensor_tensor(out=tmp[:], in0=hp[:], in1=dsb[:],
                                            op=mybir.AluOpType.mult)
                    nc.vector.tensor_reduce(
                        out=he[:, 72:80],
                        in_=tmp[:].rearrange("p (h d) -> p h d", d=8),
                        axis=mybir.AxisListType.X, op=mybir.AluOpType.add)
                    nc.sync.dma_start(out=h_ext[t * 128:(t + 1) * 128, :],
                                      in_=he[:])

            consts = {"iota16T": iota16T, "iotaP16": iotaP16, "sel8": sel8}
            tabs = {"srcP": srcP, "outP": outP, "lidsl": lidsl, "lidR": lidR}
            _phase_bins(nc, tc, consts, tabs, h_ext, out1, G, 8, 8, bsb,
                        BF16, relu=True)
    nc.compile()
    return nc


def build_l2(shapes):
    n_nodes_pad, G, S = shapes["n_pad"], shapes["G"], shapes["S"]
    T = n_nodes_pad // 128
    nc = bacc.Bacc(None)
    relu1 = nc.declare_dram_parameter("relu1", [n_nodes_pad, 128], BF16, isOutput=False)
    W2e = nc.declare_dram_parameter("W2e", [64, 42], BF16, isOutput=False)
    b2r = nc.declare_dram_parameter("b2r", [128, 40], F32, isOutput=False)
    iota16T_d = nc.declare_dram_parameter("iota16T", [128, 128], F32, isOutput=False)
    iotaP16_d = nc.declare_dram_parameter("iotaP16", [128, 128], F32, isOutput=False)
    sel8_d = nc.declare_dram_parameter("sel8", [8, 128], F32, isOutput=False)
    srcP = nc.declare_dram_parameter("srcP", [G, 128, GRP], I32, isOutput=False)
    outP = nc.declare_dram_parameter("outP", [G, 128], I32, isOutput=False)
    lidsl = nc.declare_dram_parameter("lidsl", [G, 128, 16], F32, isOutput=False)
    lidR = nc.declare_dram_parameter("lidR", [G, 1, GRP * CAP_E], F32, isOutput=False)
    out2 = nc.declare_dram_parameter("out2", [S + 16, 40], F32, isOutput=True)
    h2_ext = nc.dram_tensor("h2_ext", [n_nodes_pad, 42], F32)

    with tile.TileContext(nc) as tc:
        with tc.tile_pool(name="const", bufs=1) as cpool:
            W2sb = cpool.tile([64, 42], BF16, tag="W2sb")
            nc.sync.dma_start(out=W2sb[:], in_=W2e[:])
            bsb = cpool.tile([128, 40], F32, tag="bsb")
            nc.sync.dma_start(out=bsb[:], in_=b2r[:])
            iota16T = cpool.tile([128, 128], F32, tag="i16")
            nc.sync.dma_start(out=iota16T[:], in_=iota16T_d[:])
            iotaP16 = cpool.tile([128, 128], F32, tag="iP16")
            nc.sync.dma_start(out=iotaP16[:], in_=iotaP16_d[:])
            sel8 = cpool.tile([8, 128], F32, tag="sel8")
            nc.sync.dma_start(out=sel8[:], in_=sel8_d[:])

            with tc.tile_pool(name="pa", bufs=3) as pa, \
                 tc.tile_pool(name="pap", bufs=3, space="PSUM") as pap:
                for t in range(T):
                    r1T = pa.tile([128, 128], BF16, tag="r1T")
                    nc.sync.dma_start(out=r1T[:],
                                      in_=relu1[t * 128:(t + 1) * 128, :],
                                      transpose=True)
                    hp = pap.tile([128, 42], F32, tag="hp")
                    nc.tensor.matmul(out=hp[:], lhsT=r1T[0:64, :], rhs=W2sb[:],
                                     start=True, stop=True)
                    he = pa.tile([128, 42], F32, tag="he")
                    nc.vector.tensor_copy(out=he[:], in_=hp[:])
                    nc.sync.dma_start(out=h2_ext[t * 128:(t + 1) * 128, :],
                                      in_=he[:])

            consts = {"iota16T": iota16T, "iotaP16": iotaP16, "sel8": sel8}
            tabs = {"srcP": srcP, "outP": outP, "lidsl": lidsl, "lidR": lidR}
            _phase_bins(nc, tc, consts, tabs, h2_ext, out2, G, 1, 40, bsb,
                        F32, relu=False)
    nc.compile()
    return nc


# ----------------------------------------------------------------------------
# entry point
# ----------------------------------------------------------------------------

_CACHE = {}


def kernel(x, edge_index, W1, att_src1, att_dst1, b1, W2, att_src2, att_dst2,
           b2):
    x = np.asarray(x, np.float32)
    n_nodes = x.shape[0]
    S = n_nodes // N_CORES
    src = np.asarray(edge_index[0], np.int64).astype(np.int32)
    dst = np.asarray(edge_index[1], np.int64).astype(np.int32)

    tabs, B, G, S = _build_tables(src, dst, n_nodes, N_CORES)
    n_pad = -(-n_nodes // 128) * 128
    shapes = {"n_pad": n_pad, "G": G, "S": S}

    key = (n_nodes, G)
    if key not in _CACHE:
        _CACHE[key] = (build_l1(shapes), build_l2(shapes))
    nc1, nc2 = _CACHE[key]

    # shared constant inputs
    iota16T = np.tile(np.arange(16, dtype=np.float32), (128, 8))
    iotaP16 = np.tile((np.arange(128, dtype=np.float32) % 16)[:, None], (1, 128))
    sel8 = np.repeat(np.eye(8, dtype=np.float32), 16, axis=1)

    x_pad = np.zeros((n_pad, 128), np.float32)
    x_pad[:n_nodes] = x
    xt = np.ascontiguousarray(
        x_pad.reshape(n_pad // 128, 128, 128).transpose(0, 2, 1))

    W1 = np.asarray(W1, np.float32)
    a_s1 = np.asarray(att_src1, np.float32)
    a_d1 = np.asarray(att_dst1, np.float32)
    asrc = np.tile(a_s1.reshape(1, 64), (128, 1)).astype(np.float32)
    adst = np.tile(a_d1.reshape(1, 64), (128, 1)).astype(np.float32)
    b1r = np.tile(np.asarray(b1, np.float32).reshape(1, 64), (128, 1))

    in_maps = []
    for c in range(N_CORES):
        in_maps.append({
            "xt": xt, "W1": W1, "asrc": asrc, "adst": adst, "b1r": b1r,
            "iota16T": iota16T, "iotaP16": iotaP16, "sel8": sel8,
            "srcP": tabs["srcP"][c], "outP": tabs["outP"][c],
            "lidsl": tabs["lidsl"][c], "lidR": tabs["lidR"][c],
        })
    LAST_EXEC_NS.clear()
    res1 = _run(nc1, in_maps, list(range(N_CORES)))

    relu1 = np.zeros((n_pad, 128), ml_dtypes.bfloat16)
    for c in range(N_CORES):
        relu1[c * S:(c + 1) * S, :64] = res1.results[c]["out1"][:S]

    W2 = np.asarray(W2, np.float32)
    a_s2 = np.asarray(att_src2, np.float32).reshape(-1)
    a_d2 = np.asarray(att_dst2, np.float32).reshape(-1)
    W2e = np.concatenate([W2, (W2 @ a_s2)[:, None], (W2 @ a_d2)[:, None]],
                         axis=1).astype(ml_dtypes.bfloat16)
    b2r = np.tile(np.asarray(b2, np.float32).reshape(1, 40), (128, 1))

    in_maps2 = []
    for c in range(N_CORES):
        in_maps2.append({
            "relu1": relu1, "W2e": W2e, "b2r": b2r,
            "iota16T": iota16T, "iotaP16": iotaP16, "sel8": sel8,
            "srcP": tabs["srcP"][c], "outP": tabs["outP"][c],
            "lidsl": tabs["lidsl"][c], "lidR": tabs["lidR"][c],
        })
    res2 = _run(nc2, in_maps2, list(range(N_CORES)))

    out = np.empty((n_nodes, 40), np.float32)
    for c in range(N_CORES):
        out[c * S:(c + 1) * S] = res2.results[c]["out2"][:S]
    return out
